# revision 2
# baseline (speedup 1.0000x reference)
"""2-layer GCN encoder on two graphs, distributed over 8 Trainium2 NeuronCores.

Strategy
--------
Graph a -> cores 0-3, graph b -> cores 4-7. Each core owns 12,500 destination
nodes (relabeled for load balance) grouped into 196 ranges of 64 node slots.
Per range, in-edges are split by source-half (src < 25000 vs >=, because the
dma_gather index dtype is int16) and padded to 5 blocks of 128 edges per half.

Per 128-edge block the core gathers the 128 source rows (512 B each) with
dma_gather (SWDGE, 4 queues round-robin), builds a [128 edges x 64 dst] norm-
scaled selection matrix on the vector engine (fused is_equal*norm
tensor_scalar against a constant iota row), and accumulates
M^T @ S -> psum[feat, dst] on the tensor engine (fp32). Every destination
slot's result lands in a psum range tile; 4 ranges form a 256-column group
that flows through the dense chain W1 -> (+b) relu -> W2 on chip.

The same compiled program serves both GCN layers:
  launch A: table = x, weights (W1, b1, W2)      -> emits g = relu(xW1 agg) W2
  launch B: table = g, weights ([I|0], b2, [I;0]) -> emits relu(agg(g) + b2)
Aggregation uses the identity  A_hat (x W) == (A_hat x) W  so the sparse part
always runs at 128 features. Host does index prep / unpermute only.
"""

import os
import numpy as np

os.environ.setdefault("JAX_COMPILATION_CACHE_DIR", "/tmp/jax_cache")

import jax  # noqa: E402

try:
    jax.config.update("jax_compilation_cache_dir", "/tmp/jax_cache")
    jax.config.update("jax_persistent_cache_min_compile_time_secs", 0.0)
except Exception:
    pass

import concourse.bacc as bacc  # noqa: E402
import concourse.tile as tile  # noqa: E402
import concourse.mybir as mybir  # noqa: E402
from concourse.bass_utils import run_bass_kernel_spmd  # noqa: E402

# ---- static problem geometry (hardcoded per contract) ----
N_NODES = 50000
E_EDGES = 800000
D_IN = 128
D_HID = 256
HALF = 25000

N_CORES = 8
CORES_PER_GRAPH = 4
NODES_PER_CORE = N_NODES // CORES_PER_GRAPH  # 12500

DTILE = 64                 # dst slots per range
R = 196                    # ranges per core (196*64 = 12544 slots >= 12500)
NBLK_H = 5                 # 128-edge blocks per (range, half)
CAP_H = NBLK_H * 128       # 640 edge slots per (range, half)
NCALLS = R * 2             # one dma_gather call per (range, half)
NBLOCKS = NCALLS * NBLK_H  # 1960 S-blocks per core
IDXCOLS = CAP_H // 16      # 40 int16 columns per call
GROUPS = R // 4            # 49 dense groups of 4 ranges (256 dst slots)
MBUFS = 6                  # gather tile pool depth

META_W = DTILE + 2 * NBLOCKS  # iota | dstrel | norm

_prog = None


def _build_program():
    nc = bacc.Bacc("TRN2", target_bir_lowering=False, num_swdge_queues=4)
    f32 = mybir.dt.float32
    tbl = nc.declare_dram_parameter("tbl", [N_NODES, D_IN], f32, isOutput=False)
    idx = nc.declare_dram_parameter("idx", [128, NCALLS * IDXCOLS], mybir.dt.int16, isOutput=False)
    meta = nc.declare_dram_parameter("meta", [128, META_W], f32, isOutput=False)
    w1 = nc.declare_dram_parameter("w1", [128, D_HID], f32, isOutput=False)
    w2 = nc.declare_dram_parameter("w2", [D_HID, 128], f32, isOutput=False)
    b1c = nc.declare_dram_parameter("b1c", [128, 2], f32, isOutput=False)
    gout = nc.declare_dram_parameter("gout", [GROUPS, 128, 256], f32, isOutput=True)

    with tile.TileContext(nc) as tc:
        with (
            tc.tile_pool(name="res", bufs=1) as res,
            tc.tile_pool(name="mpool", bufs=MBUFS) as mp,
            tc.tile_pool(name="spool", bufs=4) as sp,
            tc.tile_pool(name="ssb", bufs=2) as ssb,
            tc.tile_pool(name="hsb", bufs=2) as hsb,
            tc.tile_pool(name="gsb", bufs=2) as gsbp,
            tc.tile_pool(name="psps", bufs=3, space="PSUM") as psps,
            tc.tile_pool(name="psh", bufs=2, space="PSUM") as psh,
            tc.tile_pool(name="psg", bufs=2, space="PSUM") as psg,
        ):
            idx_t = res.tile([128, NCALLS * IDXCOLS], mybir.dt.int16)
            nc.sync.dma_start(idx_t[:], idx[:, :])
            meta_t = res.tile([128, META_W], f32)
            nc.sync.dma_start(meta_t[:], meta[:, :])
            w1t = res.tile([128, D_HID], f32)
            nc.sync.dma_start(w1t[:], w1[:, :])
            w2t = res.tile([128, D_HID], f32)
            nc.sync.dma_start(w2t[:, 0:128], w2[0:128, :])
            nc.sync.dma_start(w2t[:, 128:256], w2[128:256, :])
            b1t = res.tile([128, 2], f32)
            nc.sync.dma_start(b1t[:], b1c[:, :])

            iota_ap = meta_t[:, 0:DTILE]

            for q in range(GROUPS):
                s_sb = ssb.tile([128, 256], f32, tag="s_sb")
                for rr in range(4):
                    r = q * 4 + rr
                    ps = psps.tile([128, DTILE], f32, tag="ps")
                    for h in range(2):
                        call = r * 2 + h
                        m = mp.tile([128, CAP_H], f32, tag="m")
                        nc.gpsimd.dma_gather(
                            out_ap=m[:].rearrange("p (b e) -> p b e", e=D_IN),
                            in_ap=tbl[h * HALF:(h + 1) * HALF, :],
                            idxs_ap=idx_t[:, call * IDXCOLS:(call + 1) * IDXCOLS],
                            num_idxs=CAP_H,
                            num_idxs_reg=CAP_H,
                            elem_size=D_IN,
                            single_packet=False,
                            queue_num=call % 4,
                        )
                        for b in range(NBLK_H):
                            col = call * NBLK_H + b
                            s = sp.tile([128, DTILE], f32, tag="s")
                            nc.vector.tensor_scalar(
                                out=s[:],
                                in0=iota_ap,
                                scalar1=meta_t[:, DTILE + col:DTILE + col + 1],
                                scalar2=meta_t[:, DTILE + NBLOCKS + col:DTILE + NBLOCKS + col + 1],
                                op0=mybir.AluOpType.is_equal,
                                op1=mybir.AluOpType.mult,
                            )
                            nc.tensor.matmul(
                                out=ps[:],
                                lhsT=m[:, b * 128:(b + 1) * 128],
                                rhs=s[:],
                                start=(h == 0 and b == 0),
                                stop=(h == 1 and b == NBLK_H - 1),
                            )
                    nc.vector.tensor_copy(s_sb[:, rr * DTILE:(rr + 1) * DTILE], ps[:])

                h1ps = psh.tile([128, 512], f32, tag="h1ps")
                nc.tensor.matmul(out=h1ps[:, 0:256], lhsT=w1t[:, 0:128], rhs=s_sb[:], start=True, stop=True)
                nc.tensor.matmul(out=h1ps[:, 256:512], lhsT=w1t[:, 128:256], rhs=s_sb[:], start=True, stop=True)
                h1 = hsb.tile([128, 512], f32, tag="h1")
                nc.scalar.activation(h1[:, 0:256], h1ps[:, 0:256], mybir.ActivationFunctionType.Relu, bias=b1t[:, 0:1])
                nc.scalar.activation(h1[:, 256:512], h1ps[:, 256:512], mybir.ActivationFunctionType.Relu, bias=b1t[:, 1:2])
                gps = psg.tile([128, 256], f32, tag="gps")
                nc.tensor.matmul(out=gps[:], lhsT=w2t[:, 0:128], rhs=h1[:, 0:256], start=True, stop=False)
                nc.tensor.matmul(out=gps[:], lhsT=w2t[:, 128:256], rhs=h1[:, 256:512], start=False, stop=True)
                gsb = gsbp.tile([128, 256], f32, tag="gsb")
                nc.vector.tensor_copy(gsb[:], gps[:])
                nc.sync.dma_start(gout[q], gsb[:])

    nc.compile()
    return nc


def _get_program():
    global _prog
    if _prog is None:
        _prog = _build_program()
    return _prog


def _preprocess_graph(edge):
    """Per graph: per-core packing. Returns list of 4 core dicts + dinv."""
    src = np.asarray(edge[0], np.int64)
    dst = np.asarray(edge[1], np.int64)
    deg = np.bincount(dst, minlength=N_NODES).astype(np.float32)
    dinv = (1.0 / np.sqrt(deg + np.float32(1.0))).astype(np.float32)

    # append self loops
    selfs = np.arange(N_NODES, dtype=np.int64)
    asrc = np.concatenate([src, selfs])
    adst = np.concatenate([dst, selfs])
    anorm = (dinv[asrc] * dinv[adst]).astype(np.float32)

    cores = []
    for c in range(CORES_PER_GRAPH):
        lo, hi = c * NODES_PER_CORE, (c + 1) * NODES_PER_CORE
        emask = (adst >= lo) & (adst < hi)
        es = asrc[emask]
        ed = adst[emask] - lo
        en = anorm[emask]
        eh = (es >= HALF).astype(np.int64)

        # per-node degree by half
        degh = np.zeros((NODES_PER_CORE, 2), np.int64)
        np.add.at(degh, (ed, eh), 1)

        # --- pack nodes into R bins of <=64, per-half load <= CAP_H ---
        order = np.argsort(-(degh[:, 0] + degh[:, 1]), kind="stable")
        bin_of = np.empty(NODES_PER_CORE, np.int32)
        # snake deal
        k = 0
        direction = 1
        pos = 0
        for v in order:
            bin_of[v] = pos
            k += 1
            if direction == 1:
                if pos == R - 1:
                    direction = -1
                else:
                    pos += 1
            else:
                if pos == 0:
                    direction = 1
                else:
                    pos -= 1
        # loads + capacity repair
        binload = np.zeros((R, 2), np.int64)
        np.add.at(binload, (bin_of, np.zeros_like(bin_of)), 0)
        for hh in range(2):
            np.add.at(binload[:, hh], bin_of, degh[:, hh])
        bincnt = np.bincount(bin_of, minlength=R)
        for _ in range(200):
            over = np.where((binload[:, 0] > CAP_H) | (binload[:, 1] > CAP_H))[0]
            if len(over) == 0:
                break
            bo = over[0]
            hh = 0 if binload[bo, 0] > CAP_H else 1
            # heaviest (by hh) node in bo, swap with lightest node of the
            # least-loaded bin
            cand = np.where(bin_of == bo)[0]
            vheavy = cand[np.argmax(degh[cand, hh])]
            bl = int(np.argmin(binload[:, hh]))
            cand2 = np.where(bin_of == bl)[0]
            vlight = cand2[np.argmin(degh[cand2, hh])]
            bin_of[vheavy], bin_of[vlight] = bl, bo
            for h2 in range(2):
                binload[bo, h2] += degh[vlight, h2] - degh[vheavy, h2]
                binload[bl, h2] += degh[vheavy, h2] - degh[vlight, h2]
        assert (binload <= CAP_H).all(), f"bin packing failed: {binload.max(0)}"
        assert (bincnt <= DTILE).all()

        # position of each node within its bin
        order2 = np.lexsort((np.arange(NODES_PER_CORE), bin_of))
        pos_in_bin = np.empty(NODES_PER_CORE, np.int64)
        binstart = np.zeros(R + 1, np.int64)
        np.cumsum(np.bincount(bin_of, minlength=R), out=binstart[1:])
        pos_in_bin[order2] = np.arange(NODES_PER_CORE) - binstart[bin_of[order2]]

        # column map: group q, col cidx -> global node id (or -1)
        cols_map = np.full((GROUPS, 256), -1, np.int64)
        gq = bin_of // 4
        gcol = (bin_of % 4) * DTILE + pos_in_bin
        cols_map[gq, gcol] = np.arange(lo, hi)

        # --- edge slot assembly ---
        gidx = bin_of[ed] * 2 + eh            # call index per edge
        okey = np.lexsort((np.arange(len(es)), gidx))
        gsorted = gidx[okey]
        counts = np.bincount(gsorted, minlength=NCALLS)
        assert counts.max() <= CAP_H
        starts = np.zeros(NCALLS + 1, np.int64)
        np.cumsum(counts, out=starts[1:])
        within = np.arange(len(es)) - starts[gsorted]
        slot = gsorted * CAP_H + within

        # pads gather row 0 with dstrel=-1 (S column 0 -> contribution 0);
        # every index stays valid so num_idxs_reg == valid count.
        total = NCALLS * CAP_H
        idx_slots = np.zeros(total, np.int64)
        dst_slots = np.full(total, -1.0, np.float32)
        nrm_slots = np.zeros(total, np.float32)
        idx_slots[slot] = es[okey] - eh[okey] * HALF
        dst_slots[slot] = pos_in_bin[ed[okey]].astype(np.float32)
        nrm_slots[slot] = en[okey]

        a = idx_slots.reshape(NCALLS, IDXCOLS, 16)
        idx16 = np.tile(
            np.ascontiguousarray(np.transpose(a, (2, 0, 1))).reshape(16, NCALLS * IDXCOLS),
            (8, 1),
        ).astype(np.int16)
        bblocks = dst_slots.reshape(NBLOCKS, 128)
        dstrel = np.ascontiguousarray(bblocks.T)  # [128, NBLOCKS]
        nb = nrm_slots.reshape(NBLOCKS, 128)
        norm = np.ascontiguousarray(nb.T)
        iota = np.broadcast_to(np.arange(DTILE, dtype=np.float32), (128, DTILE))
        meta = np.concatenate([iota, dstrel, norm], axis=1).astype(np.float32)

        cores.append({"idx": idx16, "meta": meta, "cols_map": cols_map})
    return cores


def _assemble(results, cores_a, cores_b):
    """Gather per-core gout into full [N, 128] arrays for each graph."""
    outs = []
    for g, cores in ((0, cores_a), (1, cores_b)):
        full = np.zeros((N_NODES, D_IN), np.float32)
        for c in range(CORES_PER_GRAPH):
            go = results[g * CORES_PER_GRAPH + c]["gout"]  # [GROUPS, 128, 256]
            cm = cores[c]["cols_map"]
            for q in range(GROUPS):
                valid = cm[q] >= 0
                full[cm[q][valid]] = go[q][:, valid].T
        outs.append(full)
    return outs


def kernel(x_a, edge_a, x_b, edge_b, W1, b1, W2, b2):
    x_a = np.ascontiguousarray(np.asarray(x_a, np.float32))
    x_b = np.ascontiguousarray(np.asarray(x_b, np.float32))
    W1 = np.asarray(W1, np.float32)
    b1 = np.asarray(b1, np.float32)
    W2 = np.asarray(W2, np.float32)
    b2 = np.asarray(b2, np.float32)

    nc = _get_program()
    cores_a = _preprocess_graph(np.asarray(edge_a))
    cores_b = _preprocess_graph(np.asarray(edge_b))

    b1c = np.stack([b1[0:128], b1[128:256]], axis=1).astype(np.float32)
    eye = np.eye(128, dtype=np.float32)
    w1_id = np.concatenate([eye, np.zeros((128, 128), np.float32)], axis=1)
    w2_id = np.concatenate([eye, np.zeros((128, 128), np.float32)], axis=0)
    b1c_id = np.stack([b2, np.zeros(128, np.float32)], axis=1).astype(np.float32)

    def maps(tbl_a, tbl_b, w1m, w2m, b1m):
        ms = []
        for g, (tbl, cores) in enumerate(((tbl_a, cores_a), (tbl_b, cores_b))):
            for c in range(CORES_PER_GRAPH):
                ms.append({
                    "tbl": tbl,
                    "idx": cores[c]["idx"],
                    "meta": cores[c]["meta"],
                    "w1": w1m, "w2": w2m, "b1c": b1m,
                })
        return ms

    core_ids = list(range(N_CORES))
    resA = run_bass_kernel_spmd(nc, maps(x_a, x_b, W1, W2, b1c), core_ids)
    g_a, g_b = _assemble(resA.results, cores_a, cores_b)
    resB = run_bass_kernel_spmd(nc, maps(g_a, g_b, w1_id, w2_id, b1c_id), core_ids)
    z_a, z_b = _assemble(resB.results, cores_a, cores_b)
    return (z_a, z_b)
